# revision 11
# baseline (speedup 1.0000x reference)
"""Conditional 1x1 conv (per-sample class-routed weights) on 8 Trainium2 cores.

Strategy (hardcoded for x:[32,64,64,512] f32, cls:[32,1] int64,
kernel:[120,1,1,512,512] f32, bias:[120,512] f32):

- Host: gather per-sample weight [B,C,F] = kernel[cls], transpose x to
  [B, C, HW] (channels-on-partitions layout so the device needs no
  transposes at all), cast x and w to bf16, shard batch 4-samples-per-core
  across 8 cores.
- Device (per core, SPMD), w-stationary orientation: for each sample and
  each 128-wide F-chunk, outT[f,pix] = sum_k w[k,f].T @ x[k,pix], the
  stationary w tile reused across 4 consecutive 512-pixel matmuls, fp32
  PSUM accumulation over the 4 contraction chunks, two 4-bank PSUM groups
  rotating so evacuation (PSUM -> SBUF bf16 cast on vector+scalar engines)
  overlaps the next group's matmuls.  Output stored transposed
  [SPC, F, NPIX] so every store DMA is a fully-contiguous-per-partition
  1 MiB transfer.  bf16 halves HBM traffic vs fp32 (the fp32 kernel was
  DMA-bound at ~234us); the bf16 kernel is PE-streaming-bound at the
  matmul roofline (~262144 PE columns/core/pass).
- Host: concat core outputs, transpose F,pix -> pix,F, upcast to fp32,
  add bias, reshape to [B,H,W,F].  bf16 keeps rel err ~3e-3, well under
  the 2e-2 gate.
"""

import numpy as np
import ml_dtypes

import concourse.bacc as bacc
import concourse.mybir as mybir
import concourse.tile as tile
from concourse import bass
from concourse.bass_utils import run_bass_kernel_spmd

BF16 = ml_dtypes.bfloat16

B, H, W, C, F = 32, 64, 64, 512, 512
NCORES = 8
SPC = B // NCORES          # samples per core
NPIX = H * W               # 4096 pixels per sample
P = 128                    # partitions
KO = C // P                # 4 contraction chunks
FO = F // P                # 4 output-channel chunks
JG = 4                     # psum tiles per group (4 x 512 pixels)
PIXG = JG * 512            # pixels per psum group
NG = NPIX // PIXG          # psum groups per F-chunk

_CACHE: dict = {}
_last_results = None       # test harness introspection


def prep_xt(xt):
    """[B or SPC, C, NPIX] fp32 -> bf16 contiguous."""
    return np.ascontiguousarray(xt.astype(BF16))


def prep_w(w):
    """[B or SPC, C, F] fp32 -> bf16 contiguous."""
    return np.ascontiguousarray(w.astype(BF16))


def _build(add_bias: bool, reps: int = 1):
    # add_bias ignored: bias is added on host (it is zeros in this problem).
    nc = bacc.Bacc("TRN2", target_bir_lowering=False, debug=False)
    xt_d = nc.declare_dram_parameter("xt", [SPC, C, NPIX], mybir.dt.bfloat16, isOutput=False)
    wt_d = nc.declare_dram_parameter("wt", [SPC, C, F], mybir.dt.bfloat16, isOutput=False)
    out_d = nc.declare_dram_parameter("out", [SPC, F, NPIX], mybir.dt.bfloat16, isOutput=True)

    XC = 512                   # x DMA chunk, pixels (0.5 MiB per chunk)

    with tile.TileContext(nc) as tc:
        with (
            tc.tile_pool(name="xpool", bufs=2) as xpool,
            tc.tile_pool(name="wpool", bufs=2) as wpool,
            tc.tile_pool(name="opool", bufs=4) as opool,
            tc.tile_pool(name="pspool", bufs=8, space="PSUM") as pspool,
        ):
          # PE pre-warm: ~12 matmuls on the first weight chunk while the
          # first x chunks stream in.  Runs once (outside the rep loop) so
          # it only affects single-shot latency: the HAM clock-gate reaches
          # 8/8 during time the PE would otherwise idle.
          warm = wpool.tile([P, 512], mybir.dt.bfloat16, tag="warm")
          nc.sync.dma_start(
              warm[:], wt_d[0].rearrange("(ko ki) f -> ki ko f", ki=P)[:, 0, :]
          )
          wps = [
              pspool.tile([P, 512], mybir.dt.float32, tag="ps", name=f"wps{i}")
              for i in range(2)
          ]
          for i in range(12):
              nc.tensor.matmul(
                  wps[i % 2][:], warm[:, :P], warm[:, :],
                  start=True, stop=True,
              )
          for _rep in range(reps):
            for s in range(SPC):
                w_sb = wpool.tile([P, KO, F], mybir.dt.bfloat16, tag="w")
                x_sb = xpool.tile([P, KO, NPIX], mybir.dt.bfloat16, tag="x")
                # f-major w chunks on HWDGE; pixel-major x chunks on SWDGE.
                # Fine-grained splits let the first matmul group start after
                # ~0.6 MiB instead of the full 4.5 MiB sample.
                for f in range(FO):
                    nc.sync.dma_start(
                        w_sb[:, :, f * P : (f + 1) * P],
                        wt_d[s].rearrange("(ko ki) f -> ki ko f", ki=P)[
                            :, :, f * P : (f + 1) * P
                        ],
                    )
                for pc in range(NPIX // XC):
                    nc.gpsimd.dma_start(
                        x_sb[:, :, pc * XC : (pc + 1) * XC],
                        xt_d[s].rearrange("(ko ki) p -> ki ko p", ki=P)[
                            :, :, pc * XC : (pc + 1) * XC
                        ],
                    )
                for g in range(NG):
                    for f in range(FO):
                        pss = [
                            pspool.tile([P, 512], mybir.dt.float32, tag="ps",
                                        name=f"ps{jj}")
                            for jj in range(JG)
                        ]
                        for k in range(KO):
                            for jj in range(JG):
                                col = g * PIXG + jj * 512
                                nc.tensor.matmul(
                                    pss[jj][:],
                                    w_sb[:, k, f * P : (f + 1) * P],
                                    x_sb[:, k, col : col + 512],
                                    start=(k == 0),
                                    stop=(k == KO - 1),
                                )
                        o_sb = opool.tile([P, PIXG], mybir.dt.bfloat16, tag="o")
                        for jj in range(JG):
                            dst = o_sb[:, jj * 512 : (jj + 1) * 512]
                            if jj % 2 == 0:
                                nc.vector.tensor_copy(out=dst, in_=pss[jj][:])
                            else:
                                nc.scalar.copy(out=dst, in_=pss[jj][:])
                        st_eng = nc.scalar if f % 2 == 0 else nc.sync
                        st_eng.dma_start(
                            out_d[s, f * P : (f + 1) * P, g * PIXG : (g + 1) * PIXG],
                            o_sb[:],
                        )
    nc.compile()
    return nc


def kernel(x, cls, kernel, bias):
    global _last_results
    x = np.ascontiguousarray(np.asarray(x, dtype=np.float32))
    cls_idx = np.asarray(cls).reshape(-1).astype(np.int64)
    ktab = np.asarray(kernel, dtype=np.float32).reshape(-1, C, F)
    bias = np.asarray(bias, dtype=np.float32)

    # host-side routing + layout prep
    w_all = prep_w(ktab[cls_idx])                           # [B, C, F] bf16
    b_all = bias[cls_idx]                                   # [B, F]
    xt_all = prep_xt(
        x.reshape(B, NPIX, C).transpose(0, 2, 1)            # [B, C, NPIX] bf16
    )

    key = "cc11bf16"
    if key not in _CACHE:
        _CACHE[key] = _build(False)
    nc = _CACHE[key]

    in_maps = []
    for c in range(NCORES):
        sl = slice(c * SPC, (c + 1) * SPC)
        in_maps.append({
            "xt": np.ascontiguousarray(xt_all[sl]),
            "wt": np.ascontiguousarray(w_all[sl]),
        })

    res = run_bass_kernel_spmd(nc, in_maps, list(range(NCORES)))
    _last_results = res

    outT = np.concatenate([res.results[c]["out"] for c in range(NCORES)], axis=0)
    out = outT.transpose(0, 2, 1).astype(np.float32)        # [B, NPIX, F]
    if np.any(b_all):
        out += b_all[:, None, :]
    return np.ascontiguousarray(out.reshape(B, H, W, F))


# revision 13
# speedup vs baseline: 1.1490x; 1.1490x over previous
"""Conditional 1x1 conv (per-sample class-routed weights) on 8 Trainium2 cores.

Strategy (hardcoded for x:[32,64,64,512] f32, cls:[32,1] int64,
kernel:[120,1,1,512,512] f32, bias:[120,512] f32):

- Host: gather per-sample weight [B,C,F] = kernel[cls], transpose x to
  [B, C, HW] (channels-on-partitions layout so the device needs no
  transposes at all), cast x and w to bf16, shard batch 4-samples-per-core
  across 8 cores.
- Device (per core, SPMD), w-stationary orientation: for each sample and
  each 128-wide F-chunk, outT[f,pix] = sum_k w[k,f].T @ x[k,pix], the
  stationary w tile reused across 4 consecutive 512-pixel matmuls, fp32
  PSUM accumulation over the 4 contraction chunks, two 4-bank PSUM groups
  rotating so evacuation (PSUM -> SBUF bf16 cast on vector+scalar engines)
  overlaps the next group's matmuls.  Output stored transposed
  [SPC, F, NPIX] so every store DMA is a fully-contiguous-per-partition
  1 MiB transfer.  bf16 halves HBM traffic vs fp32 (the fp32 kernel was
  DMA-bound at ~234us); the bf16 kernel is PE-streaming-bound at the
  matmul roofline (~262144 PE columns/core/pass).
- Host: concat core outputs, transpose F,pix -> pix,F, upcast to fp32,
  add bias, reshape to [B,H,W,F].  bf16 keeps rel err ~3e-3, well under
  the 2e-2 gate.
"""

import numpy as np
import ml_dtypes

import concourse.bacc as bacc
import concourse.mybir as mybir
import concourse.tile as tile
from concourse import bass
from concourse.bass_utils import run_bass_kernel_spmd

BF16 = ml_dtypes.bfloat16

B, H, W, C, F = 32, 64, 64, 512, 512
NCORES = 8
SPC = B // NCORES          # samples per core
NPIX = H * W               # 4096 pixels per sample
P = 128                    # partitions
KO = C // P                # 4 contraction chunks
FO = F // P                # 4 output-channel chunks
JG = 4                     # psum tiles per group (4 x 512 pixels)
PIXG = JG * 512            # pixels per psum group
NG = NPIX // PIXG          # psum groups per F-chunk

_CACHE: dict = {}
_last_results = None       # test harness introspection


def prep_xt(xt):
    """[B or SPC, C, NPIX] fp32 -> bf16 contiguous."""
    return np.ascontiguousarray(xt.astype(BF16))


def prep_w(w):
    """[B or SPC, C, F] fp32 -> bf16 contiguous."""
    return np.ascontiguousarray(w.astype(BF16))


def _build(add_bias: bool, reps: int = 1):
    # add_bias ignored: bias is added on host (it is zeros in this problem).
    nc = bacc.Bacc("TRN2", target_bir_lowering=False, debug=False)
    xt_d = nc.declare_dram_parameter("xt", [SPC, C, NPIX], mybir.dt.bfloat16, isOutput=False)
    wt_d = nc.declare_dram_parameter("wt", [SPC, C, F], mybir.dt.bfloat16, isOutput=False)
    out_d = nc.declare_dram_parameter("out", [SPC, F, NPIX], mybir.dt.bfloat16, isOutput=True)

    with tile.TileContext(nc) as tc:
        with (
            tc.tile_pool(name="xpool", bufs=2) as xpool,
            tc.tile_pool(name="wpool", bufs=2) as wpool,
            tc.tile_pool(name="opool", bufs=4) as opool,
            tc.tile_pool(name="pspool", bufs=8, space="PSUM") as pspool,
        ):
          # PE pre-warm: ~12 matmuls on the first weight chunk while the
          # first x chunks stream in.  Runs once (outside the rep loop) so
          # it only affects single-shot latency: the HAM clock-gate reaches
          # 8/8 during time the PE would otherwise idle.
          warm = wpool.tile([P, 512], mybir.dt.bfloat16, tag="warm")
          nc.sync.dma_start(
              warm[:], wt_d[0].rearrange("(ko ki) f -> ki ko f", ki=P)[:, 0, :]
          )
          wps = [
              pspool.tile([P, 512], mybir.dt.float32, tag="ps", name=f"wps{i}")
              for i in range(2)
          ]
          for i in range(12):
              nc.tensor.matmul(
                  wps[i % 2][:], warm[:, :P], warm[:, :],
                  start=True, stop=True,
              )
          for _rep in range(reps):
            for s in range(SPC):
                w_sb = wpool.tile([P, KO, F], mybir.dt.bfloat16, tag="w")
                x_sb = xpool.tile([P, KO, NPIX], mybir.dt.bfloat16, tag="x")
                # k-split DMAs: contiguous >=4 KiB runs per partition (full
                # DMA efficiency); x additionally split in pixel halves so
                # the first matmul group can start after ~2 MiB instead of
                # the full 4 MiB sample.
                for k in range(KO):
                    nc.sync.dma_start(
                        w_sb[:, k, :],
                        wt_d[s].rearrange("(ko ki) f -> ki ko f", ki=P)[:, k, :],
                    )
                for h in range(2):
                    for k in range(KO):
                        cols = slice(h * (NPIX // 2), (h + 1) * (NPIX // 2))
                        nc.gpsimd.dma_start(
                            x_sb[:, k, cols],
                            xt_d[s].rearrange("(ko ki) p -> ki ko p", ki=P)[:, k, cols],
                        )
                for f in range(FO):
                    o_sb = opool.tile([P, NPIX], mybir.dt.bfloat16, tag="o")
                    for g in range(NG):
                        pss = [
                            pspool.tile([P, 512], mybir.dt.float32, tag="ps",
                                        name=f"ps{jj}")
                            for jj in range(JG)
                        ]
                        for k in range(KO):
                            for jj in range(JG):
                                col = g * PIXG + jj * 512
                                nc.tensor.matmul(
                                    pss[jj][:],
                                    w_sb[:, k, f * P : (f + 1) * P],
                                    x_sb[:, k, col : col + 512],
                                    start=(k == 0),
                                    stop=(k == KO - 1),
                                )
                        for jj in range(JG):
                            col = g * PIXG + jj * 512
                            dst = o_sb[:, col : col + 512]
                            if jj % 2 == 0:
                                nc.vector.tensor_copy(out=dst, in_=pss[jj][:])
                            else:
                                nc.scalar.copy(out=dst, in_=pss[jj][:])
                    st_eng = nc.scalar if f % 2 == 0 else nc.sync
                    st_eng.dma_start(out_d[s, f * P : (f + 1) * P, :], o_sb[:])
    nc.compile()
    return nc


def kernel(x, cls, kernel, bias):
    global _last_results
    x = np.ascontiguousarray(np.asarray(x, dtype=np.float32))
    cls_idx = np.asarray(cls).reshape(-1).astype(np.int64)
    ktab = np.asarray(kernel, dtype=np.float32).reshape(-1, C, F)
    bias = np.asarray(bias, dtype=np.float32)

    # host-side routing + layout prep
    w_all = prep_w(ktab[cls_idx])                           # [B, C, F] bf16
    b_all = bias[cls_idx]                                   # [B, F]
    xt_all = prep_xt(
        x.reshape(B, NPIX, C).transpose(0, 2, 1)            # [B, C, NPIX] bf16
    )

    key = "cc11bf16"
    if key not in _CACHE:
        _CACHE[key] = _build(False)
    nc = _CACHE[key]

    in_maps = []
    for c in range(NCORES):
        sl = slice(c * SPC, (c + 1) * SPC)
        in_maps.append({
            "xt": np.ascontiguousarray(xt_all[sl]),
            "wt": np.ascontiguousarray(w_all[sl]),
        })

    res = run_bass_kernel_spmd(nc, in_maps, list(range(NCORES)))
    _last_results = res

    outT = np.concatenate([res.results[c]["out"] for c in range(NCORES)], axis=0)
    out = outT.transpose(0, 2, 1).astype(np.float32)        # [B, NPIX, F]
    if np.any(b_all):
        out += b_all[:, None, :]
    return np.ascontiguousarray(out.reshape(B, H, W, F))


# revision 15
# speedup vs baseline: 1.3937x; 1.2130x over previous
"""Conditional 1x1 conv (per-sample class-routed weights) on 8 Trainium2 cores.

Strategy (hardcoded for x:[32,64,64,512] f32, cls:[32,1] int64,
kernel:[120,1,1,512,512] f32, bias:[120,512] f32):

- Host: gather per-sample weight [B,C,F] = kernel[cls], transpose x to
  [B, C, HW] (channels-on-partitions layout so the device needs no
  transposes at all), cast x and w to bf16, shard batch 4-samples-per-core
  across 8 cores.
- Device (per core, SPMD), w-stationary orientation: for each sample and
  each 128-wide F-chunk, outT[f,pix] = sum_k w[k,f].T @ x[k,pix], the
  stationary w tile reused across 4 consecutive 512-pixel matmuls, fp32
  PSUM accumulation over the 4 contraction chunks, two 4-bank PSUM groups
  rotating so evacuation (PSUM -> SBUF bf16 cast on vector+scalar engines)
  overlaps the next group's matmuls.  Output stored transposed
  [SPC, F, NPIX] so every store DMA is a fully-contiguous-per-partition
  1 MiB transfer.  bf16 halves HBM traffic vs fp32 (the fp32 kernel was
  DMA-bound at ~234us); the bf16 kernel is PE-streaming-bound at the
  matmul roofline (~262144 PE columns/core/pass).
- Host: concat core outputs, transpose F,pix -> pix,F, upcast to fp32,
  add bias, reshape to [B,H,W,F].  bf16 keeps rel err ~3e-3, well under
  the 2e-2 gate.
"""

import numpy as np
import ml_dtypes

import concourse.bacc as bacc
import concourse.mybir as mybir
import concourse.tile as tile
from concourse import bass
from concourse.bass_utils import run_bass_kernel_spmd

BF16 = ml_dtypes.bfloat16

B, H, W, C, F = 32, 64, 64, 512, 512
NCORES = 8
SPC = B // NCORES          # samples per core
NPIX = H * W               # 4096 pixels per sample
P = 128                    # partitions
KO = C // P                # 4 contraction chunks
FO = F // P                # 4 output-channel chunks
JG = 4                     # psum tiles per group (4 x 512 pixels)
PIXG = JG * 512            # pixels per psum group
NG = NPIX // PIXG          # psum groups per F-chunk

_CACHE: dict = {}
_last_results = None       # test harness introspection


def prep_xt(xt):
    """[B or SPC, C, NPIX] fp32 -> bf16 contiguous."""
    return np.ascontiguousarray(xt.astype(BF16))


def prep_w(w):
    """[B or SPC, C, F] fp32 -> bf16 contiguous."""
    return np.ascontiguousarray(w.astype(BF16))


def _build(add_bias: bool, reps: int = 1):
    # add_bias ignored: bias is added on host (it is zeros in this problem).
    nc = bacc.Bacc("TRN2", target_bir_lowering=False, debug=False)
    xt_d = nc.declare_dram_parameter("xt", [SPC, C, NPIX], mybir.dt.bfloat16, isOutput=False)
    wt_d = nc.declare_dram_parameter("wt", [SPC, C, F], mybir.dt.bfloat16, isOutput=False)
    out_d = nc.declare_dram_parameter("out", [SPC, F, NPIX], mybir.dt.bfloat16, isOutput=True)

    with tile.TileContext(nc) as tc:
        with (
            tc.tile_pool(name="xpool", bufs=2) as xpool,
            tc.tile_pool(name="wpool", bufs=2) as wpool,
            tc.tile_pool(name="opool", bufs=4) as opool,
            tc.tile_pool(name="pspool", bufs=8, space="PSUM") as pspool,
        ):
          # PE pre-warm: ~12 matmuls on the first weight chunk while the
          # first x chunks stream in.  Runs once (outside the rep loop) so
          # it only affects single-shot latency: the HAM clock-gate reaches
          # 8/8 during time the PE would otherwise idle.
          warm = wpool.tile([P, 512], mybir.dt.bfloat16, tag="warm")
          nc.sync.dma_start(
              warm[:], wt_d[0].rearrange("(ko ki) f -> ki ko f", ki=P)[:, 0, :]
          )
          wps = [
              pspool.tile([P, 512], mybir.dt.float32, tag="ps", name=f"wps{i}")
              for i in range(2)
          ]
          for i in range(12):
              nc.tensor.matmul(
                  wps[i % 2][:], warm[:, :P], warm[:, :],
                  start=True, stop=True,
              )
          for _rep in range(reps):
            for s in range(SPC):
                w_sb = wpool.tile([P, KO, F], mybir.dt.bfloat16, tag="w")
                x_sb = xpool.tile([P, KO, NPIX], mybir.dt.bfloat16, tag="x")
                # k-split DMAs: contiguous >=4 KiB runs per partition (full
                # DMA efficiency); x additionally split in pixel halves so
                # the first matmul group can start after ~2 MiB instead of
                # the full 4 MiB sample.
                for k in range(KO):
                    nc.sync.dma_start(
                        w_sb[:, k, :],
                        wt_d[s].rearrange("(ko ki) f -> ki ko f", ki=P)[:, k, :],
                    )
                for h in range(2):
                    for k in range(KO):
                        cols = slice(h * (NPIX // 2), (h + 1) * (NPIX // 2))
                        nc.gpsimd.dma_start(
                            x_sb[:, k, cols],
                            xt_d[s].rearrange("(ko ki) p -> ki ko p", ki=P)[:, k, cols],
                        )
                for f in range(FO):
                    o_sb = opool.tile([P, NPIX], mybir.dt.bfloat16, tag="o")
                    for g in range(NG):
                        pss = [
                            pspool.tile([P, 512], mybir.dt.float32, tag="ps",
                                        name=f"ps{jj}")
                            for jj in range(JG)
                        ]
                        for k in range(KO):
                            for jj in range(JG):
                                col = g * PIXG + jj * 512
                                nc.tensor.matmul(
                                    pss[jj][:],
                                    w_sb[:, k, f * P : (f + 1) * P],
                                    x_sb[:, k, col : col + 512],
                                    start=(k == 0),
                                    stop=(k == KO - 1),
                                )
                        for jj in range(JG):
                            col = g * PIXG + jj * 512
                            dst = o_sb[:, col : col + 512]
                            if jj % 2 == 0:
                                nc.vector.tensor_copy(out=dst, in_=pss[jj][:])
                            else:
                                nc.scalar.copy(out=dst, in_=pss[jj][:])
                    st_eng = nc.scalar if f % 2 == 0 else nc.sync
                    st_eng.dma_start(out_d[s, f * P : (f + 1) * P, :], o_sb[:])
    nc.compile()
    return nc


def kernel(x, cls, kernel, bias):
    global _last_results
    x = np.ascontiguousarray(np.asarray(x, dtype=np.float32))
    cls_idx = np.asarray(cls).reshape(-1).astype(np.int64)
    ktab = np.asarray(kernel, dtype=np.float32).reshape(-1, C, F)
    bias = np.asarray(bias, dtype=np.float32)

    # host-side routing + layout prep
    w_all = prep_w(ktab[cls_idx])                           # [B, C, F] bf16
    b_all = bias[cls_idx]                                   # [B, F]
    xt_all = prep_xt(
        x.reshape(B, NPIX, C).transpose(0, 2, 1)            # [B, C, NPIX] bf16
    )

    key = "cc11bf16"
    if key not in _CACHE:
        _CACHE[key] = _build(False)
    nc = _CACHE[key]

    in_maps = []
    for c in range(NCORES):
        sl = slice(c * SPC, (c + 1) * SPC)
        in_maps.append({
            "xt": np.ascontiguousarray(xt_all[sl]),
            "wt": np.ascontiguousarray(w_all[sl]),
        })

    res = run_bass_kernel_spmd(nc, in_maps, list(range(NCORES)))
    _last_results = res

    outT = np.concatenate([res.results[c]["out"] for c in range(NCORES)], axis=0)
    out = outT.transpose(0, 2, 1).astype(np.float32)        # [B, NPIX, F]
    if np.any(b_all):
        out += b_all[:, None, :]
    return np.ascontiguousarray(out.reshape(B, H, W, F))
